# revision 1
# baseline (speedup 1.0000x reference)
"""Per-sample dynamic conv2d (VALID) on 8 Trainium2 NeuronCores.

Problem: X [32,128,128,128] f32 (NHWC), kernel [32,3,3,128,128] f32 (per-sample
HWIO) -> out [32,126,126,128] f32.

Sharding: pure data-parallel over batch; each of the 8 cores runs 4 samples.

Per-core algorithm (per sample b):
  1. Transpose X[b] to channel-major XT [Cin, H*W] via PE transposes
     (batched row loads -> PE transpose (4 per PSUM bank) -> one wide DVE
     copy-cast per bank).
  2. For each input row r and kw in {0,1,2}: matmul
       P_r[w', (kh,co)] += XT[:, r*128+kw : +128].T @ Kkw[:, (kh,co)]
     where Kkw = kernel[b, :, kw] laid out [Cin, 3*Cout], accumulating the
     3 kw taps in PSUM (N=384 keeps the PE at full rate for f32r).
  3. out[h'] = sum_kh P_{h'+kh}[:, kh*128:(kh+1)*128]: ACT seed copy + two
     DVE adds into a 6-row batch tile, then one DMA stores 6 NHWC rows.

Columns w'>=126 of each P tile are convolution overrun garbage and are never
read.  A post-Tile pass splits semaphore waits >1 per instruction onto NoOps
(walrus codegen allows only one sync-wait on self-loading f32/f32r matmuls
and few on drains).
"""

import numpy as np

import concourse.bass as bass
import concourse.mybir as mybir
from concourse.bass_utils import run_bass_kernel_spmd
from concourse.masks import make_identity
from concourse.tile import TileContext

N_CORES = 8
B, H, W, C = 32, 128, 128, 128
KK = 3
BL = B // N_CORES            # samples per core
HO = WO = H - KK + 1         # 126
XT_PAD = H * W + 128         # padded free size; weights read up to H*W+2
LROWS = 16                   # input rows per load DMA
SROWS = 6                    # output rows per store DMA (126 = 21*6)

F32 = mybir.dt.float32
F32R = mybir.dt.float32r
BF16 = mybir.dt.bfloat16

MODE = "f32r"                # "f32r" (rel err ~1.6e-4) or "bf16" (~2e-3, faster PE)


def _split_excess_waits(nc, limit=1):
    """walrus codegen rejects >1 sync-wait on several instruction kinds
    (self-loading f32/f32r Matmult, Drain).  Move excess waits onto
    preceding same-engine NoOps."""
    n = 0
    for bb in nc.m.functions[0].blocks:
        out = []
        changed = False
        for inst in bb.instructions:
            si = inst.sync_info
            if si is not None and len(si.on_wait) > limit:
                waits = list(si.on_wait)
                excess, keep = waits[:-limit], waits[-limit:]
                for i in range(0, len(excess), limit):
                    n += 1
                    out.append(
                        mybir.InstNoOp(
                            name=f"I-waitsplit-{n}",
                            engine=inst.engine,
                            bass_nofuse=True,
                            sync_info=mybir.SyncInfo(
                                on_wait=excess[i : i + limit], on_update=[]
                            ),
                        )
                    )
                inst.sync_info = mybir.SyncInfo(on_wait=keep, on_update=si.on_update)
                changed = True
            out.append(inst)
        if changed:
            bb.instructions = out
    return n


def _build(mode=MODE):
    xdt = F32R if mode == "f32r" else BF16  # staging/transpose dtype (f32r
    # streams PE transposes at 1.5 cycles/row vs f32's 2.0, bits preserved)
    mdt = F32R if mode == "f32r" else BF16  # matmul operand dtype

    nc = bass.Bass()
    Xd = nc.declare_dram_parameter("X", [BL, H, W, C], F32, isOutput=False)
    Kd = nc.declare_dram_parameter("kern", [BL, KK, KK, C, C], F32, isOutput=False)
    Od = nc.declare_dram_parameter("out", [BL, HO, WO, C], F32, isOutput=True)

    with TileContext(nc) as tc:
        with (
            tc.tile_pool(name="const", bufs=1) as p_const,
            tc.tile_pool(name="xt", bufs=2) as p_xt,
            tc.tile_pool(name="stage", bufs=3) as p_stage,
            tc.tile_pool(name="ktap", bufs=2) as p_k,
            tc.tile_pool(name="outb", bufs=4) as p_out,
            tc.tile_pool(name="pst", bufs=2, space="PSUM") as p_pst,
            tc.tile_pool(name="pacc", bufs=6, space="PSUM") as p_pacc,
        ):
            identf = p_const.tile([128, 128], F32, tag="identf")
            make_identity(nc, identf[:, :])
            if xdt == F32:
                ident = identf
            else:
                ident = p_const.tile([128, 128], xdt, tag="ident")
                nc.vector.tensor_copy(ident[:, :], identf[:, :])

            HH = H // 2

            def emit_T(b):
                """Yield thunks that emit sample b's load/transpose/cast phase
                piecewise, so it can be interleaved into the previous sample's
                matmul phase (keeps PE/DVE queues dense across samples)."""
                ktiles = []
                for kw in range(KK):
                    kt = p_k.tile([C, KK * C], mdt, tag=f"ktap{kw}")
                    nc.gpsimd.dma_start(
                        out=kt[:, :].rearrange("p (kh co) -> p kh co", kh=KK),
                        in_=Kd[b, :, kw].rearrange("kh ci co -> ci kh co"),
                    )
                    ktiles.append(kt)
                xt_lo = p_xt.tile([C, (HH + 1) * 128], mdt, tag="xtlo")
                xt_hi = p_xt.tile([C, (HH + 1) * 128], mdt, tag="xthi")
                state = {"ktiles": ktiles, "lo": xt_lo, "hi": xt_hi}

                if b == 0:
                    ranges = [(0, 4), (4, 16)] + [
                        (h0, h0 + LROWS) for h0 in range(16, H, LROWS)
                    ]
                else:
                    ranges = [(h0, h0 + LROWS) for h0 in range(0, H, LROWS)]

                def thunks():
                    for h0, h1 in ranges:
                        def load(h0=h0, h1=h1):
                            xr = p_stage.tile([W, LROWS * C], xdt, tag="xrow")
                            dma = nc.sync if xdt == F32 else nc.gpsimd
                            dma.dma_start(
                                out=xr[:, : (h1 - h0) * C].rearrange(
                                    "w (h c) -> w h c", h=h1 - h0
                                ),
                                in_=Xd[b, h0:h1].rearrange("h w c -> w h c"),
                            )
                            state["xr"] = xr
                        yield load
                        for q in range(0, h1 - h0, 4):
                            def ptgrp(h0=h0, q=q):
                                xr = state["xr"]
                                pt = p_pst.tile([C, 4 * W], xdt, tag="tp")
                                for i in range(4):
                                    nc.tensor.transpose(
                                        pt[:, i * 128 : (i + 1) * 128],
                                        xr[:, (q + i) * 128 : (q + i + 1) * 128],
                                        ident[:, :],
                                    )
                                h = h0 + q
                                if h < HH:
                                    nc.vector.tensor_copy(
                                        xt_lo[:, h * 128 : (h + 4) * 128], pt[:, :]
                                    )
                                else:
                                    nc.vector.tensor_copy(
                                        xt_hi[:, (h - HH) * 128 : (h - HH + 4) * 128],
                                        pt[:, :],
                                    )
                                    if h == HH:
                                        # matmuls at r=HH-1, kw>0 read 2 cols
                                        # of row HH
                                        nc.vector.tensor_copy(
                                            xt_lo[:, HH * 128 : (HH + 1) * 128],
                                            pt[:, 0:128],
                                        )
                            yield ptgrp

                state["thunks"] = thunks()
                return state

            def emit_M(b, st, nxt):
                """Emit sample b's matmul/reduce/store phase, interleaving the
                next sample's T-phase thunks (if any) every few rows."""
                ktiles, xt_lo, xt_hi = st["ktiles"], st["lo"], st["hi"]
                live = {}
                ot = None
                for r in range(H):
                    if nxt is not None and r % 3 == 0:
                        for t in (next(nxt["thunks"], None),):
                            if t is not None:
                                t()
                    pr = p_pacc.tile([W, KK * C], F32, tag="P")
                    xth, rl = (xt_lo, r) if r < HH else (xt_hi, r - HH)
                    for kw in range(KK):
                        nc.tensor.matmul(
                            pr[:, :],
                            xth[:, rl * 128 + kw : rl * 128 + kw + 128],
                            ktiles[kw][:, :],
                            start=(kw == 0),
                            stop=(kw == KK - 1),
                        )
                    live[r] = pr
                    if r >= KK - 1:
                        hp = r - (KK - 1)       # output row
                        j = hp % SROWS
                        if j == 0:
                            ot = p_out.tile([W, SROWS * C], F32, tag="ot")
                        seg = slice(j * C, (j + 1) * C)
                        # DVE tensor_tensor may read only one PSUM input;
                        # seed on ACT, then two DVE adds (SBUF+PSUM each).
                        nc.scalar.copy(ot[0:WO, seg], live[hp][0:WO, 0:C])
                        nc.vector.tensor_add(
                            ot[0:WO, seg],
                            ot[0:WO, seg],
                            live[hp + 1][0:WO, C : 2 * C],
                        )
                        nc.vector.tensor_add(
                            ot[0:WO, seg],
                            ot[0:WO, seg],
                            live[hp + 2][0:WO, 2 * C : 3 * C],
                        )
                        del live[hp]
                        if j == SROWS - 1:
                            g = hp - j
                            nc.sync.dma_start(
                                out=Od[b, g : g + SROWS].rearrange(
                                    "h w c -> w h c"
                                ),
                                in_=ot[0:WO, :].rearrange(
                                    "w (h c) -> w h c", h=SROWS
                                ),
                            )
                if nxt is not None:
                    for t in nxt["thunks"]:
                        t()

            st = emit_T(0)
            for t in st["thunks"]:
                t()
            st["thunks"] = iter(())
            for b in range(BL):
                nxt = emit_T(b + 1) if b + 1 < BL else None
                emit_M(b, st, nxt)
                st = nxt

    _split_excess_waits(nc)
    return nc


_CACHE = {}


def _get_nc():
    if "nc" not in _CACHE:
        _CACHE["nc"] = _build()
    return _CACHE["nc"]


def _run(X, kern, **kw):
    in_maps = [
        {
            "X": np.ascontiguousarray(X[c * BL : (c + 1) * BL]),
            "kern": np.ascontiguousarray(kern[c * BL : (c + 1) * BL]),
        }
        for c in range(N_CORES)
    ]
    last_err = None
    for _attempt in range(3):
        try:
            res = run_bass_kernel_spmd(
                _get_nc(), in_maps, list(range(N_CORES)), **kw
            )
            break
        except Exception as e:  # transient NRT_EXEC_UNIT_UNRECOVERABLE etc.
            last_err = e
    else:
        raise last_err
    out = np.concatenate([res.results[c]["out"] for c in range(N_CORES)], axis=0)
    return out, res


def kernel(X, kernel):
    X = np.ascontiguousarray(X, dtype=np.float32)
    kern = np.ascontiguousarray(kernel, dtype=np.float32)
    out, _ = _run(X, kern)
    return out



# revision 2
# speedup vs baseline: 1.8851x; 1.8851x over previous
"""Per-sample dynamic conv2d (VALID) on 8 Trainium2 NeuronCores.

Problem: X [32,128,128,128] f32 (NHWC), kernel [32,3,3,128,128] f32 (per-sample
HWIO) -> out [32,126,126,128] f32.

Sharding: pure data-parallel over batch; each of the 8 cores runs 4 samples.

Per-core algorithm (per sample b):
  1. Transpose X[b] to channel-major XT [Cin, H*W] via PE transposes
     (batched row loads -> PE transpose (4 per PSUM bank) -> one wide DVE
     copy-cast per bank).
  2. For each input row r and kw in {0,1,2}: matmul
       P_r[w', (kh,co)] += XT[:, r*128+kw : +128].T @ Kkw[:, (kh,co)]
     where Kkw = kernel[b, :, kw] laid out [Cin, 3*Cout], accumulating the
     3 kw taps in PSUM (N=384 keeps the PE at full rate for f32r).
  3. out[h'] = sum_kh P_{h'+kh}[:, kh*128:(kh+1)*128]: ACT seed copy + two
     DVE adds into a 6-row batch tile, then one DMA stores 6 NHWC rows.

Columns w'>=126 of each P tile are convolution overrun garbage and are never
read.  A post-Tile pass splits semaphore waits >1 per instruction onto NoOps
(walrus codegen allows only one sync-wait on self-loading f32/f32r matmuls
and few on drains).
"""

import numpy as np

import concourse.bass as bass
import concourse.mybir as mybir
from concourse.bass_utils import run_bass_kernel_spmd
from concourse.masks import make_identity
from concourse.tile import TileContext

N_CORES = 8
B, H, W, C = 32, 128, 128, 128
KK = 3
BL = B // N_CORES            # samples per core
HO = WO = H - KK + 1         # 126
XT_PAD = H * W + 128         # padded free size; weights read up to H*W+2
LROWS = 16                   # input rows per load DMA
SROWS = 6                    # output rows per store DMA (126 = 21*6)

F32 = mybir.dt.float32
F32R = mybir.dt.float32r
BF16 = mybir.dt.bfloat16

MODE = "bf16"                # "f32r" (rel err ~1.6e-4) or "bf16" (~2e-3, faster PE)


def _split_excess_waits(nc, limit=1):
    """walrus codegen rejects >1 sync-wait on several instruction kinds
    (self-loading f32/f32r Matmult, Drain).  Move excess waits onto
    preceding same-engine NoOps."""
    n = 0
    for bb in nc.m.functions[0].blocks:
        out = []
        changed = False
        for inst in bb.instructions:
            si = inst.sync_info
            if si is not None and len(si.on_wait) > limit:
                waits = list(si.on_wait)
                excess, keep = waits[:-limit], waits[-limit:]
                for i in range(0, len(excess), limit):
                    n += 1
                    out.append(
                        mybir.InstNoOp(
                            name=f"I-waitsplit-{n}",
                            engine=inst.engine,
                            bass_nofuse=True,
                            sync_info=mybir.SyncInfo(
                                on_wait=excess[i : i + limit], on_update=[]
                            ),
                        )
                    )
                inst.sync_info = mybir.SyncInfo(on_wait=keep, on_update=si.on_update)
                changed = True
            out.append(inst)
        if changed:
            bb.instructions = out
    return n


def _build(mode=MODE):
    xdt = F32R if mode == "f32r" else BF16  # staging/transpose dtype (f32r
    # streams PE transposes at 1.5 cycles/row vs f32's 2.0, bits preserved)
    mdt = F32R if mode == "f32r" else BF16  # matmul operand dtype

    nc = bass.Bass()
    Xd = nc.declare_dram_parameter("X", [BL, H, W, C], F32, isOutput=False)
    Kd = nc.declare_dram_parameter("kern", [BL, KK, KK, C, C], F32, isOutput=False)
    Od = nc.declare_dram_parameter("out", [BL, HO, WO, C], F32, isOutput=True)

    with TileContext(nc) as tc:
        with (
            tc.tile_pool(name="const", bufs=1) as p_const,
            tc.tile_pool(name="xt", bufs=2) as p_xt,
            tc.tile_pool(name="stage", bufs=3) as p_stage,
            tc.tile_pool(name="ktap", bufs=2) as p_k,
            tc.tile_pool(name="outb", bufs=4) as p_out,
            tc.tile_pool(name="pst", bufs=2, space="PSUM") as p_pst,
            tc.tile_pool(name="pacc", bufs=6, space="PSUM") as p_pacc,
        ):
            identf = p_const.tile([128, 128], F32, tag="identf")
            make_identity(nc, identf[:, :])
            if xdt == F32:
                ident = identf
            else:
                ident = p_const.tile([128, 128], xdt, tag="ident")
                nc.vector.tensor_copy(ident[:, :], identf[:, :])

            HH = H // 2

            def emit_T(b):
                """Yield thunks that emit sample b's load/transpose/cast phase
                piecewise, so it can be interleaved into the previous sample's
                matmul phase (keeps PE/DVE queues dense across samples)."""
                ktiles = []
                for kw in range(KK):
                    kt = p_k.tile([C, KK * C], mdt, tag=f"ktap{kw}")
                    nc.gpsimd.dma_start(
                        out=kt[:, :].rearrange("p (kh co) -> p kh co", kh=KK),
                        in_=Kd[b, :, kw].rearrange("kh ci co -> ci kh co"),
                    )
                    ktiles.append(kt)
                xt_lo = p_xt.tile([C, (HH + 1) * 128], mdt, tag="xtlo")
                xt_hi = p_xt.tile([C, (HH + 1) * 128], mdt, tag="xthi")
                state = {"ktiles": ktiles, "lo": xt_lo, "hi": xt_hi}

                if b == 0:
                    ranges = [(0, 4), (4, 16)] + [
                        (h0, h0 + LROWS) for h0 in range(16, H, LROWS)
                    ]
                else:
                    ranges = [(h0, h0 + LROWS) for h0 in range(0, H, LROWS)]

                def thunks():
                    for h0, h1 in ranges:
                        def load(h0=h0, h1=h1):
                            xr = p_stage.tile([W, LROWS * C], xdt, tag="xrow")
                            dma = nc.sync if xdt == F32 else nc.gpsimd
                            dma.dma_start(
                                out=xr[:, : (h1 - h0) * C].rearrange(
                                    "w (h c) -> w h c", h=h1 - h0
                                ),
                                in_=Xd[b, h0:h1].rearrange("h w c -> w h c"),
                            )
                            state["xr"] = xr
                        yield load
                        for q in range(0, h1 - h0, 4):
                            def ptgrp(h0=h0, q=q):
                                xr = state["xr"]
                                pt = p_pst.tile([C, 4 * W], xdt, tag="tp")
                                for i in range(4):
                                    nc.tensor.transpose(
                                        pt[:, i * 128 : (i + 1) * 128],
                                        xr[:, (q + i) * 128 : (q + i + 1) * 128],
                                        ident[:, :],
                                    )
                                h = h0 + q
                                if h < HH:
                                    nc.vector.tensor_copy(
                                        xt_lo[:, h * 128 : (h + 4) * 128], pt[:, :]
                                    )
                                else:
                                    nc.vector.tensor_copy(
                                        xt_hi[:, (h - HH) * 128 : (h - HH + 4) * 128],
                                        pt[:, :],
                                    )
                                    if h == HH:
                                        # matmuls at r=HH-1, kw>0 read 2 cols
                                        # of row HH
                                        nc.vector.tensor_copy(
                                            xt_lo[:, HH * 128 : (HH + 1) * 128],
                                            pt[:, 0:128],
                                        )
                            yield ptgrp

                state["thunks"] = thunks()
                return state

            def emit_M(b, st, nxt):
                """Emit sample b's matmul/reduce/store phase, interleaving the
                next sample's T-phase thunks (if any) every few rows."""
                ktiles, xt_lo, xt_hi = st["ktiles"], st["lo"], st["hi"]
                live = {}
                ot = None
                for r in range(H):
                    if nxt is not None and r % 3 == 0:
                        for t in (next(nxt["thunks"], None),):
                            if t is not None:
                                t()
                    pr = p_pacc.tile([W, KK * C], F32, tag="P")
                    xth, rl = (xt_lo, r) if r < HH else (xt_hi, r - HH)
                    for kw in range(KK):
                        nc.tensor.matmul(
                            pr[:, :],
                            xth[:, rl * 128 + kw : rl * 128 + kw + 128],
                            ktiles[kw][:, :],
                            start=(kw == 0),
                            stop=(kw == KK - 1),
                        )
                    live[r] = pr
                    if r >= KK - 1:
                        hp = r - (KK - 1)       # output row
                        j = hp % SROWS
                        if j == 0:
                            ot = p_out.tile([W, SROWS * C], F32, tag="ot")
                        seg = slice(j * C, (j + 1) * C)
                        # DVE tensor_tensor may read only one PSUM input;
                        # seed on ACT, then two DVE adds (SBUF+PSUM each).
                        nc.scalar.copy(ot[0:WO, seg], live[hp][0:WO, 0:C])
                        nc.vector.tensor_add(
                            ot[0:WO, seg],
                            ot[0:WO, seg],
                            live[hp + 1][0:WO, C : 2 * C],
                        )
                        nc.vector.tensor_add(
                            ot[0:WO, seg],
                            ot[0:WO, seg],
                            live[hp + 2][0:WO, 2 * C : 3 * C],
                        )
                        del live[hp]
                        if j == SROWS - 1:
                            g = hp - j
                            nc.sync.dma_start(
                                out=Od[b, g : g + SROWS].rearrange(
                                    "h w c -> w h c"
                                ),
                                in_=ot[0:WO, :].rearrange(
                                    "w (h c) -> w h c", h=SROWS
                                ),
                            )
                if nxt is not None:
                    for t in nxt["thunks"]:
                        t()

            st = emit_T(0)
            for t in st["thunks"]:
                t()
            st["thunks"] = iter(())
            for b in range(BL):
                nxt = emit_T(b + 1) if b + 1 < BL else None
                emit_M(b, st, nxt)
                st = nxt

    _split_excess_waits(nc)
    return nc


_CACHE = {}


def _get_nc():
    if "nc" not in _CACHE:
        _CACHE["nc"] = _build()
    return _CACHE["nc"]


def _run(X, kern, **kw):
    in_maps = [
        {
            "X": np.ascontiguousarray(X[c * BL : (c + 1) * BL]),
            "kern": np.ascontiguousarray(kern[c * BL : (c + 1) * BL]),
        }
        for c in range(N_CORES)
    ]
    last_err = None
    for _attempt in range(3):
        try:
            res = run_bass_kernel_spmd(
                _get_nc(), in_maps, list(range(N_CORES)), **kw
            )
            break
        except Exception as e:  # transient NRT_EXEC_UNIT_UNRECOVERABLE etc.
            last_err = e
    else:
        raise last_err
    out = np.concatenate([res.results[c]["out"] for c in range(N_CORES)], axis=0)
    return out, res


def kernel(X, kernel):
    X = np.ascontiguousarray(X, dtype=np.float32)
    kern = np.ascontiguousarray(kernel, dtype=np.float32)
    out, _ = _run(X, kern)
    return out



# revision 3
# speedup vs baseline: 1.9324x; 1.0251x over previous
"""Per-sample dynamic conv2d (VALID) on 8 Trainium2 NeuronCores — v3.

v3 = v2 (bf16 upload, DMA-xbar transpose loads, 9-tap PSUM accumulation)
with quad row-packing: each 2KB PSUM bank holds FOUR consecutive output rows
(4 x 128 f32).  The kernel taps are host-rearranged to [kw, j=2-kh] order so
that for input row r and kw, the taps of consecutive target rows are
CONTIGUOUS 128-col blocks in SBUF: one matmul with N=128*nrows covers a whole
run of rows inside a quad (psum cols (hp-4q)*128...).  This roughly halves
the matmul instruction count (same streamed columns) and evacuates four rows
per copy instead of one.

Quad q (rows 4q..4q+3) opens at (r=4q, kw=0) with start=True and closes at
(r=4q+5, kw=2) with stop=True (q=31 holds rows 124-125, closes at r=127).
All evacuations run on DVE (the PE's start-matmuls wait on them via bank
recycling, and the DVE queue never carries DMA-lane waits); stores batch 6
quads (24 rows) as bf16 on the ACT HWDGE ring (host casts the output back to
f32), keeping the SP ring free for the transpose loads.  ot bufs=6 decouples
evacuations from store completions (which queue behind transpose traffic on
the shared SDMA engines).
"""

import numpy as np
import ml_dtypes

import concourse.bass as bass
import concourse.mybir as mybir
from concourse.bass_utils import run_bass_kernel_spmd
from concourse.tile import TileContext

N_CORES = 8
B, H, W, C = 32, 128, 128, 128
KK = 3
BL = B // N_CORES            # samples per core
HO = WO = H - KK + 1         # 126
HW = H * W
XT_PAD = HW + 128            # matmuls read up to HW+2
NQ = (HO + 3) // 4           # 32 quads (last holds 2 rows)
QG = 6                       # quads per store group (24 rows; 6 stores/sample)

F32 = mybir.dt.float32
BF16 = mybir.dt.bfloat16


def _split_excess_waits(nc, limit=1):
    """walrus codegen rejects >1 sync-wait on several instruction kinds.
    Move excess waits onto preceding same-engine NoOps."""
    n = 0
    for bb in nc.m.functions[0].blocks:
        out = []
        changed = False
        for inst in bb.instructions:
            si = inst.sync_info
            if si is not None and len(si.on_wait) > limit:
                waits = list(si.on_wait)
                excess, keep = waits[:-limit], waits[-limit:]
                for i in range(0, len(excess), limit):
                    n += 1
                    out.append(
                        mybir.InstNoOp(
                            name=f"I-waitsplit-{n}",
                            engine=inst.engine,
                            bass_nofuse=True,
                            sync_info=mybir.SyncInfo(
                                on_wait=excess[i : i + limit], on_update=[]
                            ),
                        )
                    )
                inst.sync_info = mybir.SyncInfo(on_wait=keep, on_update=si.on_update)
                changed = True
            out.append(inst)
        if changed:
            bb.instructions = out
    return n


def _build():
    nc = bass.Bass()
    Xd = nc.declare_dram_parameter("X", [BL, HW, C], BF16, isOutput=False)
    # host-rearranged: t = kw*3 + j with j = 2-kh
    Kd = nc.declare_dram_parameter("kern", [BL, KK * KK, C, C], BF16, isOutput=False)
    Od = nc.declare_dram_parameter("out", [BL, HO, WO, C], BF16, isOutput=True)

    with TileContext(nc) as tc:
        with (
            tc.tile_pool(name="xt", bufs=3) as p_xt,
            tc.tile_pool(name="kt", bufs=3) as p_k,
            tc.tile_pool(name="outb", bufs=6) as p_out,
            tc.tile_pool(name="pacc", bufs=8, space="PSUM") as p_acc,
        ):
            def emit_load(b, nchunks):
                xt = p_xt.tile([C, XT_PAD], BF16, tag="xt")
                step = HW // nchunks
                for c0 in range(0, HW, step):
                    nc.sync.dma_start(
                        out=xt[:, c0 : c0 + step],
                        in_=Xd[b, c0 : c0 + step, :],
                        transpose=True,
                    )
                # small; SWDGE ring is idle — keeps it off the transpose FIFO
                kall = p_k.tile([C, KK * KK * C], BF16, tag="kall")
                nc.gpsimd.dma_start(
                    out=kall[:, :].rearrange("ci (t co) -> ci t co", t=KK * KK),
                    in_=Kd[b].rearrange("t ci co -> ci t co"),
                )
                return {"kall": kall, "xt": xt}

            def emit_compute(b, st):
                kall, xt = st["kall"], st["xt"]
                live = {}
                ot = None
                for r in range(H):
                    for kw in range(KK):
                        x_sl = xt[:, r * 128 + kw : r * 128 + kw + 128]
                        a, hp_hi = max(0, r - 2), min(r, HO - 1)
                        while a <= hp_hi:
                            q = a // 4
                            b_end = min(hp_hi, 4 * q + 3)
                            nt = b_end - a + 1
                            if q not in live:
                                pr = p_acc.tile([W, 512], F32, tag="P")
                                live[q] = pr
                            j_a = a - r + 2
                            close_r = 4 * q + 5 if q < NQ - 1 else H - 1
                            nc.tensor.matmul(
                                live[q][:, (a - 4 * q) * C : (a - 4 * q + nt) * C],
                                x_sl,
                                kall[:, (kw * KK + j_a) * C : (kw * KK + j_a + nt) * C],
                                start=(r == 4 * q and kw == 0),
                                stop=(r == close_r and kw == KK - 1),
                            )
                            a = b_end + 1
                    # quads closing at this r
                    closed = []
                    if r >= 5 and (r - 5) % 4 == 0:
                        closed.append((r - 5) // 4)
                    if r == H - 1:
                        closed.append(NQ - 1)
                    for q in closed:
                        nrows = min(4, HO - 4 * q)
                        g = q // QG          # store group
                        k = q % QG           # quad slot within group
                        if k == 0:
                            ot = p_out.tile([WO, QG * 4 * C], BF16, tag="ot")
                        src = live.pop(q)[0:WO, 0 : nrows * C]
                        dst = ot[0:WO, k * 4 * C : (k * 4 + nrows) * C]
                        # ALL evacs on DVE: the PE's start-matmuls wait on
                        # these via bank recycling, and the DVE queue never
                        # carries DMA-lane waits (unlike ACT, whose store
                        # DMAs wait on lanes recycled from the transposes).
                        nc.vector.tensor_copy(dst, src)
                        last_in_group = (k == QG - 1) or (q == NQ - 1)
                        if last_in_group:
                            base = g * QG * 4
                            nr = k * 4 + nrows
                            nc.scalar.dma_start(
                                out=Od[b, base : base + nr].rearrange(
                                    "h w c -> w h c"
                                ),
                                in_=ot[0:WO, 0 : nr * C].rearrange(
                                    "w (h c) -> w h c", h=nr
                                ),
                            )

            # 2 loads + 6 stores per sample = 8 HWDGE DMAs, matching the 8
            # DMAHW completion-sem lanes Tile round-robins: each DMA's
            # lane-recycling wait lands on the same DMA kind one sample
            # back, so loads never gate on stores (which would make them
            # just-in-time instead of prefetched).
            st = emit_load(0, nchunks=8)
            for b in range(BL):
                nxt = emit_load(b + 1, nchunks=2) if b + 1 < BL else None
                emit_compute(b, st)
                st = nxt

    _split_excess_waits(nc)
    return nc


_CACHE = {}


def _get_nc():
    if "nc" not in _CACHE:
        _CACHE["nc"] = _build()
    return _CACHE["nc"]


def _run(X, kern, **kw):
    Xb = X.astype(ml_dtypes.bfloat16).reshape(B, HW, C)
    # [B, kh, kw, ci, co] -> [B, kw, j=2-kh, ci, co] -> [B, 9, ci, co]
    Kb = (
        kern.astype(ml_dtypes.bfloat16)[:, ::-1]
        .transpose(0, 2, 1, 3, 4)
        .reshape(B, KK * KK, C, C)
    )
    in_maps = [
        {
            "X": np.ascontiguousarray(Xb[c * BL : (c + 1) * BL]),
            "kern": np.ascontiguousarray(Kb[c * BL : (c + 1) * BL]),
        }
        for c in range(N_CORES)
    ]
    last_err = None
    for _attempt in range(3):
        try:
            res = run_bass_kernel_spmd(
                _get_nc(), in_maps, list(range(N_CORES)), **kw
            )
            break
        except Exception as e:  # transient NRT_EXEC_UNIT_UNRECOVERABLE etc.
            last_err = e
    else:
        raise last_err
    out = np.concatenate(
        [np.asarray(res.results[c]["out"]).astype(np.float32) for c in range(N_CORES)],
        axis=0,
    )
    return out, res


def kernel(X, kernel):
    X = np.ascontiguousarray(X, dtype=np.float32)
    kern = np.ascontiguousarray(kernel, dtype=np.float32)
    out, _ = _run(X, kern)
    return out
